# revision 30
# baseline (speedup 1.0000x reference)
"""Additive-attention kernel for Trainium2, 8-way T-sharded SPMD.

Math (per reference):
  h[b,t,:]  = values[b,t,:] @ W1.T + b1 + query[b,:] @ W2.T + b2        [B,T,64]
  score     = tanh(h) @ Wv.T (+ bv, dropped: softmax shift-invariant)   [B,T,1]
  aw        = softmax(score, axis=0)   (over B)
  context   = sum_b aw * values                                         [T,256]
  returns (context, aw)

Sharding: T (dim 1) split across 8 cores; B stays whole per core so the
softmax over B is core-local. No collectives.

Per-core layout strategy: values arrive b-partitioned [128b, t, 256v]
(cast-DMA f32->bf16).  The W1 contraction needs v on partitions, so each
[128b,128v] tile is TE-transposed (bf16 PSUM) and copied to SBUF.  h is
produced transposed [64h, 128b] with two t's stacked via col-group
tile_position, so tanh runs on full 128 partitions and the Wv reduction
over h becomes an M=2 matmul (Wv stacked twice) giving score rows per
t-pair.  exp+sum(Z) via one ACT op per score bank; weighted sum is M=1
matmuls with the aw column as stationary, accumulating over b in PSUM.
"""
import sys
sys.path.insert(0, '/opt/trn_rl_repo')
import numpy as np
import ml_dtypes

import concourse.mybir as mybir
from concourse import bacc, tile
from concourse.bass_utils import run_bass_kernel_spmd

DT = mybir.dt
BF = ml_dtypes.bfloat16

B, T, V, Q, H = 512, 512, 256, 128, 64
NCORES = 8
TS = T // NCORES          # 64 t's per core
TG = 16                   # t's per group (one softmax/wsum phase)
NG = TS // TG             # 4 groups
NBT = B // 128            # 4 b-tiles

_nc_cache = None
LAST_RESULT = None


def build():
    nc = bacc.Bacc(None, target_bir_lowering=False)

    vals = nc.dram_tensor("vals", [B, TS, V], DT.float32, kind="ExternalInput")
    queryT = nc.dram_tensor("queryT", [Q, B], DT.bfloat16, kind="ExternalInput")
    w1t = nc.dram_tensor("w1t", [128, 2, H], DT.bfloat16, kind="ExternalInput")
    w2t = nc.dram_tensor("w2t", [Q, H], DT.bfloat16, kind="ExternalInput")
    bsum2 = nc.dram_tensor("bsum2", [128, 1], DT.float32, kind="ExternalInput")
    wv2 = nc.dram_tensor("wv2", [128, 2], DT.bfloat16, kind="ExternalInput")

    ctx_out = nc.dram_tensor("ctx", [TS, V], DT.float32, kind="ExternalOutput")
    awt_out = nc.dram_tensor("awt", [TS, B], DT.bfloat16, kind="ExternalOutput")

    with tile.TileContext(nc) as tc:
        with (
            tc.tile_pool(name="const", bufs=1) as cpool,
            tc.tile_pool(name="vnat", bufs=10) as vnat_pool,
            tc.tile_pool(name="vtb", bufs=2) as vtb_pool,
            tc.tile_pool(name="work", bufs=4) as work,
            tc.tile_pool(name="soft", bufs=4) as soft,
            tc.tile_pool(name="awp", bufs=10) as awp,
            tc.tile_pool(name="ps_tr", bufs=2, space="PSUM") as ps_tr,
            tc.tile_pool(name="ps_h", bufs=2, space="PSUM") as ps_h,
            tc.tile_pool(name="ps_sc", bufs=2, space="PSUM") as ps_sc,
            tc.tile_pool(name="ps_cx", bufs=1, space="PSUM") as ps_cx,
        ):
            # ---- constants ----
            ones_sb = cpool.tile([128, 128], DT.bfloat16)
            nc.vector.memset(ones_sb[:], 1.0)
            id_sb = cpool.tile([128, 128], DT.bfloat16)
            nc.gpsimd.affine_select(id_sb[:], ones_sb[:], [[1, 128]],
                                    mybir.AluOpType.is_equal, 0.0,
                                    base=0, channel_multiplier=-1)
            w1t_sb = cpool.tile([128, 2, H], DT.bfloat16)
            nc.sync.dma_start(w1t_sb[:], w1t[:])
            w2t_sb = cpool.tile([Q, H], DT.bfloat16)
            nc.sync.dma_start(w2t_sb[:], w2t[:])
            wv2_sb = cpool.tile([128, 2], DT.bfloat16)
            nc.sync.dma_start(wv2_sb[:], wv2[:])
            bsum2_sb = cpool.tile([128, 1], DT.float32)
            nc.sync.dma_start(bsum2_sb[:], bsum2[:])
            qT_sb = cpool.tile([Q, B], DT.bfloat16)
            nc.sync.dma_start(qT_sb[:], queryT[:])

            # q_proj^T = W2T.T @ queryT -> [64h, 512b], stacked x2 into qb
            qp_ps = ps_h.tile([H, B], DT.float32, tag="h")
            nc.tensor.matmul(qp_ps[:], w2t_sb[:], qT_sb[:], start=True, stop=True)
            qb_sb = cpool.tile([128, B], DT.float32)
            nc.vector.tensor_copy(qb_sb[0:H, :], qp_ps[:])
            nc.vector.tensor_copy(qb_sb[H:128, :], qp_ps[:])

            vnat = {}
            copy_alt = [0]
            for g in range(NG):
                # ============ phase 1: transposes for whole t-group ============
                vtb = vtb_pool.tile([128, 2 * TG, B], DT.bfloat16, tag="vtb",
                                    name=f"vtb{g}")
                for bt in range(NBT):
                    vn = vnat_pool.tile([128, TG, V], DT.bfloat16, tag="vn",
                                        name=f"vn{g}_{bt}")
                    vnat[(g, bt)] = vn
                    nsp = 4 if (g == 0 and bt == 0) else 2
                    step = TG // nsp
                    for hf in range(nsp):
                        nc.gpsimd.dma_start(
                            vn[:, hf * step:(hf + 1) * step, :],
                            vals[bt * 128:(bt + 1) * 128,
                                 g * TG + hf * step:g * TG + (hf + 1) * step, :])
                    for q4 in range(4):
                        trp = ps_tr.tile([128, 8, 128], DT.bfloat16, tag="tr",
                                         name=f"tr{g}_{bt}_{q4}")
                        for k in range(4):
                            tl = q4 * 4 + k
                            for vt in range(2):
                                nc.tensor.transpose(
                                    trp[:, 2 * k + vt, :],
                                    vn[:, tl, vt * 128:(vt + 1) * 128],
                                    id_sb[:])
                        dst = vtb[:, 8 * q4:8 * q4 + 8, bt * 128:(bt + 1) * 128]
                        if copy_alt[0] % 2 == 1:
                            nc.scalar.activation(
                                dst, trp[:], mybir.ActivationFunctionType.Copy)
                        else:
                            nc.vector.tensor_copy(dst, trp[:])
                        copy_alt[0] += 1

                # ============ dense h / score bursts (N=512) ============
                scp = [ps_sc.tile([128, B], DT.float32, tag="sc", name=f"sc{g}_{i}")
                       for i in range(2)]
                hps = [ps_h.tile([128, B], DT.float32, tag="h", name=f"h{g}_{i}")
                       for i in range(TG // 2)]
                for kt in range(2):
                    for r in range(2):
                        for tp in range(TG // 2):
                            tl = 2 * tp + r
                            nc.tensor.matmul(
                                hps[tp][r * 64:(r + 1) * 64, :],
                                w1t_sb[:, kt, :],
                                vtb[:, 2 * tl + kt, :],
                                start=(kt == 0), stop=(kt == 1),
                                tile_position=(0, r * 64))
                for tp in range(TG // 2):
                    hs = work.tile([128, B], DT.float32, tag="hs",
                                   name=f"hs{g}_{tp}")
                    nc.vector.tensor_tensor(
                        hs[:], hps[tp][:], qb_sb[:], mybir.AluOpType.add)
                    th = work.tile([128, B], DT.bfloat16, tag="th",
                                   name=f"th{g}_{tp}")
                    nc.scalar.activation(
                        th[:], hs[:], mybir.ActivationFunctionType.Tanh,
                        bias=bsum2_sb[:])
                    po = 32 * (tp % 4)
                    nc.tensor.matmul(
                        scp[tp // 4][po:po + 2, :], wv2_sb[:], th[:],
                        start=True, stop=True, tile_position=(0, po))

                # ============ phase 2: softmax + weighted sum ============
                awts = []
                for ab in range(2):
                    ew = soft.tile([128, B], DT.float32, tag="ew", name=f"ew{g}_{ab}")
                    z = soft.tile([128, 1], DT.float32, tag="z", name=f"z{g}_{ab}")
                    nc.scalar.activation(
                        ew[:], scp[ab][:], mybir.ActivationFunctionType.Exp,
                        accum_out=z[:])
                    rz = soft.tile([128, 1], DT.float32, tag="rz", name=f"rz{g}_{ab}")
                    nc.vector.reciprocal(rz[:], z[:])
                    ewn = soft.tile([128, B], DT.bfloat16, tag="ewn", name=f"ewn{g}_{ab}")
                    nc.vector.tensor_scalar_mul(ewn[:], ew[:], rz[:])
                    s = g * TG + ab * 8
                    nc.sync.dma_start(awt_out[s:s + 8:2, :], ewn[0:97:32, :])
                    nc.sync.dma_start(awt_out[s + 1:s + 8:2, :], ewn[1:98:32, :])
                    for bt in range(NBT):
                        awp_ps = ps_tr.tile([128, 128], DT.bfloat16, tag="tr",
                                            name=f"awp{g}_{ab}_{bt}")
                        nc.tensor.transpose(
                            awp_ps[:], ewn[:, bt * 128:(bt + 1) * 128], id_sb[:])
                        aw_sb = awp.tile([128, 128], DT.bfloat16, tag="aw",
                                         name=f"aw{g}_{ab}_{bt}")
                        nc.vector.tensor_copy(aw_sb[:], awp_ps[:])
                        awts.append(aw_sb)

                cxp = ps_cx.tile([128, 4, V], DT.float32, tag="cx", name=f"cx{g}")
                for tl in range(TG):
                    ab, p4, r = tl // 8, (tl % 8) // 2, tl % 2
                    col = 32 * p4 + r
                    q_, cr = divmod(tl, 4)
                    for bt in range(NBT):
                        nc.tensor.matmul(
                            cxp[32 * cr:32 * cr + 1, q_, :],
                            awts[ab * 4 + bt][:, col:col + 1],
                            vnat[(g, bt)][:, tl, :],
                            start=(bt == 0), stop=(bt == NBT - 1),
                            tile_position=(0, 32 * cr))
                cx_sb = work.tile([128, 4, V], DT.float32, tag="cxs", name=f"cxs{g}")
                nc.vector.tensor_copy(cx_sb[:], cxp[:])
                for cr in range(4):
                    nc.sync.dma_start(
                        ctx_out[g * TG + cr:g * TG + cr + 13:4, :],
                        cx_sb[32 * cr:32 * cr + 1, :, :])
    nc.compile()
    return nc


def _host_prep(query, W1, b1, W2, b2, Wv):
    queryT = np.ascontiguousarray(query.T).astype(BF)
    w1t = np.zeros((128, 2, H), dtype=BF)
    for vt in range(2):
        w1t[:, vt, :] = W1[:, vt * 128:(vt + 1) * 128].T.astype(BF)
    w2t = np.ascontiguousarray(W2.T).astype(BF)
    bsum2 = np.tile((b1 + b2).astype(np.float32), 2).reshape(128, 1)
    wv2 = np.zeros((128, 2), dtype=BF)
    wv2[0:64, 0] = Wv[0].astype(BF)
    wv2[64:128, 1] = Wv[0].astype(BF)
    return queryT, w1t, w2t, np.ascontiguousarray(bsum2), wv2


def kernel(query, values, W1, b1, W2, b2, Wv, bv):
    global _nc_cache, LAST_RESULT
    query = np.asarray(query, np.float32)
    values = np.asarray(values, np.float32)
    if _nc_cache is None:
        _nc_cache = build()
    nc = _nc_cache
    queryT, w1t, w2t, bsum2, wv2 = _host_prep(
        np.asarray(query), np.asarray(W1), np.asarray(b1),
        np.asarray(W2), np.asarray(b2), np.asarray(Wv))
    in_maps = []
    for c in range(NCORES):
        shard = np.ascontiguousarray(values[:, c * TS:(c + 1) * TS, :])
        in_maps.append(dict(vals=shard, queryT=queryT, w1t=w1t, w2t=w2t,
                            bsum2=bsum2, wv2=wv2))
    res = run_bass_kernel_spmd(nc, in_maps, core_ids=list(range(NCORES)))
    LAST_RESULT = res
    context = np.empty((T, V), np.float32)
    aw = np.empty((B, T, 1), np.float32)
    for c in range(NCORES):
        context[c * TS:(c + 1) * TS, :] = res.results[c]["ctx"]
        aw[:, c * TS:(c + 1) * TS, 0] = res.results[c]["awt"].astype(np.float32).T
    return context, aw


# revision 31
# speedup vs baseline: 1.0862x; 1.0862x over previous
"""Additive-attention kernel for Trainium2, 8-way T-sharded SPMD.

Math (per reference):
  h[b,t,:]  = values[b,t,:] @ W1.T + b1 + query[b,:] @ W2.T + b2        [B,T,64]
  score     = tanh(h) @ Wv.T (+ bv, dropped: softmax shift-invariant)   [B,T,1]
  aw        = softmax(score, axis=0)   (over B)
  context   = sum_b aw * values                                         [T,256]
  returns (context, aw)

Sharding: T (dim 1) split across 8 cores; B stays whole per core so the
softmax over B is core-local. No collectives.

Per-core layout strategy: values arrive b-partitioned [128b, t, 256v]
(cast-DMA f32->bf16).  The W1 contraction needs v on partitions, so each
[128b,128v] tile is TE-transposed (bf16 PSUM) and copied to SBUF.  h is
produced transposed [64h, 128b] with two t's stacked via col-group
tile_position, so tanh runs on full 128 partitions and the Wv reduction
over h becomes an M=2 matmul (Wv stacked twice) giving score rows per
t-pair.  exp+sum(Z) via one ACT op per score bank; weighted sum is M=1
matmuls with the aw column as stationary, accumulating over b in PSUM.
"""
import sys
sys.path.insert(0, '/opt/trn_rl_repo')
import numpy as np
import ml_dtypes

import concourse.mybir as mybir
from concourse import bacc, tile
from concourse.bass_utils import run_bass_kernel_spmd

DT = mybir.dt
BF = ml_dtypes.bfloat16

B, T, V, Q, H = 512, 512, 256, 128, 64
NCORES = 8
TS = T // NCORES          # 64 t's per core
TG = 16                   # t's per group (one softmax/wsum phase)
NG = TS // TG             # 4 groups
NBT = B // 128            # 4 b-tiles

_nc_cache = None
LAST_RESULT = None


def build():
    nc = bacc.Bacc(None, target_bir_lowering=False)

    vals = nc.dram_tensor("vals", [B, TS, V], DT.float32, kind="ExternalInput")
    queryT = nc.dram_tensor("queryT", [Q, B], DT.bfloat16, kind="ExternalInput")
    w1t = nc.dram_tensor("w1t", [128, 2, H], DT.bfloat16, kind="ExternalInput")
    w2t = nc.dram_tensor("w2t", [Q, H], DT.bfloat16, kind="ExternalInput")
    bsum2 = nc.dram_tensor("bsum2", [128, 1], DT.float32, kind="ExternalInput")
    wv2 = nc.dram_tensor("wv2", [128, 2], DT.bfloat16, kind="ExternalInput")

    ctx_out = nc.dram_tensor("ctx", [TS, V], DT.float32, kind="ExternalOutput")
    awt_out = nc.dram_tensor("awt", [TS, B], DT.bfloat16, kind="ExternalOutput")

    with tile.TileContext(nc) as tc:
        with (
            tc.tile_pool(name="const", bufs=1) as cpool,
            tc.tile_pool(name="vnat", bufs=10) as vnat_pool,
            tc.tile_pool(name="vtb", bufs=2) as vtb_pool,
            tc.tile_pool(name="work", bufs=4) as work,
            tc.tile_pool(name="soft", bufs=4) as soft,
            tc.tile_pool(name="awp", bufs=10) as awp,
            tc.tile_pool(name="ps_tr", bufs=2, space="PSUM") as ps_tr,
            tc.tile_pool(name="ps_h", bufs=2, space="PSUM") as ps_h,
            tc.tile_pool(name="ps_sc", bufs=2, space="PSUM") as ps_sc,
            tc.tile_pool(name="ps_cx", bufs=1, space="PSUM") as ps_cx,
        ):
            # ---- constants ----
            ones_sb = cpool.tile([128, 128], DT.bfloat16)
            nc.vector.memset(ones_sb[:], 1.0)
            id_sb = cpool.tile([128, 128], DT.bfloat16)
            nc.gpsimd.affine_select(id_sb[:], ones_sb[:], [[1, 128]],
                                    mybir.AluOpType.is_equal, 0.0,
                                    base=0, channel_multiplier=-1)
            w1t_sb = cpool.tile([128, 2, H], DT.bfloat16)
            nc.sync.dma_start(w1t_sb[:], w1t[:])
            w2t_sb = cpool.tile([Q, H], DT.bfloat16)
            nc.sync.dma_start(w2t_sb[:], w2t[:])
            wv2_sb = cpool.tile([128, 2], DT.bfloat16)
            nc.sync.dma_start(wv2_sb[:], wv2[:])
            bsum2_sb = cpool.tile([128, 1], DT.float32)
            nc.sync.dma_start(bsum2_sb[:], bsum2[:])
            qT_sb = cpool.tile([Q, B], DT.bfloat16)
            nc.sync.dma_start(qT_sb[:], queryT[:])

            # q_proj^T = W2T.T @ queryT -> [64h, 512b], stacked x2 into qb
            qp_ps = ps_h.tile([H, B], DT.float32, tag="h")
            nc.tensor.matmul(qp_ps[:], w2t_sb[:], qT_sb[:], start=True, stop=True)
            qb_sb = cpool.tile([128, B], DT.float32)
            nc.vector.tensor_copy(qb_sb[0:H, :], qp_ps[:])
            nc.vector.tensor_copy(qb_sb[H:128, :], qp_ps[:])

            vnat = {}
            copy_alt = [0]
            for g in range(NG):
                # ============ phase 1: transposes for whole t-group ============
                vtb = vtb_pool.tile([128, 2 * TG, B], DT.bfloat16, tag="vtb",
                                    name=f"vtb{g}")
                for bt in range(NBT):
                    vn = vnat_pool.tile([128, TG, V], DT.bfloat16, tag="vn",
                                        name=f"vn{g}_{bt}")
                    vnat[(g, bt)] = vn
                    nsp = 4 if (g == 0 and bt == 0) else 2
                    step = TG // nsp
                    for hf in range(nsp):
                        nc.gpsimd.dma_start(
                            vn[:, hf * step:(hf + 1) * step, :],
                            vals[bt * 128:(bt + 1) * 128,
                                 g * TG + hf * step:g * TG + (hf + 1) * step, :])
                    for q4 in range(4):
                        trp = ps_tr.tile([128, 8, 128], DT.bfloat16, tag="tr",
                                         name=f"tr{g}_{bt}_{q4}")
                        for k in range(4):
                            tl = q4 * 4 + k
                            for vt in range(2):
                                nc.tensor.transpose(
                                    trp[:, 2 * k + vt, :],
                                    vn[:, tl, vt * 128:(vt + 1) * 128],
                                    id_sb[:])
                        dst = vtb[:, 8 * q4:8 * q4 + 8, bt * 128:(bt + 1) * 128]
                        if copy_alt[0] % 3 == 2:
                            nc.scalar.activation(
                                dst, trp[:], mybir.ActivationFunctionType.Copy)
                        else:
                            nc.vector.tensor_copy(dst, trp[:])
                        copy_alt[0] += 1

                # ============ dense h / score bursts (N=512) ============
                scp = [ps_sc.tile([128, B], DT.float32, tag="sc", name=f"sc{g}_{i}")
                       for i in range(2)]
                hps = [ps_h.tile([128, B], DT.float32, tag="h", name=f"h{g}_{i}")
                       for i in range(TG // 2)]
                for kt in range(2):
                    for r in range(2):
                        for tp in range(TG // 2):
                            tl = 2 * tp + r
                            nc.tensor.matmul(
                                hps[tp][r * 64:(r + 1) * 64, :],
                                w1t_sb[:, kt, :],
                                vtb[:, 2 * tl + kt, :],
                                start=(kt == 0), stop=(kt == 1),
                                tile_position=(0, r * 64))
                for tp in range(TG // 2):
                    hs = work.tile([128, B], DT.float32, tag="hs",
                                   name=f"hs{g}_{tp}")
                    nc.vector.tensor_tensor(
                        hs[:], hps[tp][:], qb_sb[:], mybir.AluOpType.add)
                    th = work.tile([128, B], DT.bfloat16, tag="th",
                                   name=f"th{g}_{tp}")
                    nc.scalar.activation(
                        th[:], hs[:], mybir.ActivationFunctionType.Tanh,
                        bias=bsum2_sb[:])
                    po = 32 * (tp % 4)
                    nc.tensor.matmul(
                        scp[tp // 4][po:po + 2, :], wv2_sb[:], th[:],
                        start=True, stop=True, tile_position=(0, po))

                # ============ phase 2: softmax + weighted sum ============
                awts = []
                for ab in range(2):
                    ew = soft.tile([128, B], DT.float32, tag="ew", name=f"ew{g}_{ab}")
                    z = soft.tile([128, 1], DT.float32, tag="z", name=f"z{g}_{ab}")
                    nc.scalar.activation(
                        ew[:], scp[ab][:], mybir.ActivationFunctionType.Exp,
                        accum_out=z[:])
                    rz = soft.tile([128, 1], DT.float32, tag="rz", name=f"rz{g}_{ab}")
                    nc.vector.reciprocal(rz[:], z[:])
                    ewn = soft.tile([128, B], DT.bfloat16, tag="ewn", name=f"ewn{g}_{ab}")
                    nc.vector.tensor_scalar_mul(ewn[:], ew[:], rz[:])
                    s = g * TG + ab * 8
                    nc.sync.dma_start(awt_out[s:s + 8:2, :], ewn[0:97:32, :])
                    nc.sync.dma_start(awt_out[s + 1:s + 8:2, :], ewn[1:98:32, :])
                    for bt in range(NBT):
                        awp_ps = ps_tr.tile([128, 128], DT.bfloat16, tag="tr",
                                            name=f"awp{g}_{ab}_{bt}")
                        nc.tensor.transpose(
                            awp_ps[:], ewn[:, bt * 128:(bt + 1) * 128], id_sb[:])
                        aw_sb = awp.tile([128, 128], DT.bfloat16, tag="aw",
                                         name=f"aw{g}_{ab}_{bt}")
                        nc.vector.tensor_copy(aw_sb[:], awp_ps[:])
                        awts.append(aw_sb)

                cxp = ps_cx.tile([128, 4, V], DT.float32, tag="cx", name=f"cx{g}")
                for tl in range(TG):
                    ab, p4, r = tl // 8, (tl % 8) // 2, tl % 2
                    col = 32 * p4 + r
                    q_, cr = divmod(tl, 4)
                    for bt in range(NBT):
                        nc.tensor.matmul(
                            cxp[32 * cr:32 * cr + 1, q_, :],
                            awts[ab * 4 + bt][:, col:col + 1],
                            vnat[(g, bt)][:, tl, :],
                            start=(bt == 0), stop=(bt == NBT - 1),
                            tile_position=(0, 32 * cr))
                cx_sb = work.tile([128, 4, V], DT.float32, tag="cxs", name=f"cxs{g}")
                nc.vector.tensor_copy(cx_sb[:], cxp[:])
                for cr in range(4):
                    nc.sync.dma_start(
                        ctx_out[g * TG + cr:g * TG + cr + 13:4, :],
                        cx_sb[32 * cr:32 * cr + 1, :, :])
    nc.compile()
    return nc


def _host_prep(query, W1, b1, W2, b2, Wv):
    queryT = np.ascontiguousarray(query.T).astype(BF)
    w1t = np.zeros((128, 2, H), dtype=BF)
    for vt in range(2):
        w1t[:, vt, :] = W1[:, vt * 128:(vt + 1) * 128].T.astype(BF)
    w2t = np.ascontiguousarray(W2.T).astype(BF)
    bsum2 = np.tile((b1 + b2).astype(np.float32), 2).reshape(128, 1)
    wv2 = np.zeros((128, 2), dtype=BF)
    wv2[0:64, 0] = Wv[0].astype(BF)
    wv2[64:128, 1] = Wv[0].astype(BF)
    return queryT, w1t, w2t, np.ascontiguousarray(bsum2), wv2


def kernel(query, values, W1, b1, W2, b2, Wv, bv):
    global _nc_cache, LAST_RESULT
    query = np.asarray(query, np.float32)
    values = np.asarray(values, np.float32)
    if _nc_cache is None:
        _nc_cache = build()
    nc = _nc_cache
    queryT, w1t, w2t, bsum2, wv2 = _host_prep(
        np.asarray(query), np.asarray(W1), np.asarray(b1),
        np.asarray(W2), np.asarray(b2), np.asarray(Wv))
    in_maps = []
    for c in range(NCORES):
        shard = np.ascontiguousarray(values[:, c * TS:(c + 1) * TS, :])
        in_maps.append(dict(vals=shard, queryT=queryT, w1t=w1t, w2t=w2t,
                            bsum2=bsum2, wv2=wv2))
    res = run_bass_kernel_spmd(nc, in_maps, core_ids=list(range(NCORES)))
    LAST_RESULT = res
    context = np.empty((T, V), np.float32)
    aw = np.empty((B, T, 1), np.float32)
    for c in range(NCORES):
        context[c * TS:(c + 1) * TS, :] = res.results[c]["ctx"]
        aw[:, c * TS:(c + 1) * TS, 0] = res.results[c]["awt"].astype(np.float32).T
    return context, aw
